# revision 1
# baseline (speedup 1.0000x reference)
"""Block-sparse (DeepSpeed fixed-layout) self-attention on 8 Trainium2 cores.

Strategy
--------
Shard the 32 (batch, head) slices across 8 cores (4 each, pure data parallel).
For each slice, queries are processed in windows of 128 rows (4 key-blocks of
32). The union of active key blocks for a window is split into "chunk slots"
of up to 128 keys; slots are deduplicated across windows (the fixed layout's
global stripe makes most slots shared). Host-side numpy pre-gathers:
  QT  [S, 64, L]            query transposed (hd on partitions)
  KTg [S, 64, nch*128]      gathered+transposed key chunks
  Vg  [S, 128, nch, 65]     gathered value chunks with a ones column
On device, per window and chunk:
  S_T  = KT_chunk.T-matmul (scores arrive keys-on-partitions: no P transpose)
  P    = exp(scale * S_T)   on ACT, straight from PSUM
  mask: memset invalid (key-block, query-block) cells to zero
  O~  += P.T @ [V | 1]      accumulated in PSUM; col 64 = softmax denominator
then O = O~[:, :64] * (1 / O~[:, 64]) and DMA out. exp() needs no max
subtraction: scores are ~N(0,1) after scaling, far from fp32 overflow.
"""

import sys

sys.path.insert(0, "/opt/trn_rl_repo")

import numpy as np

N_CORES = 8

# dtype knobs: storage+matmul dtype for scores (QT/KTg) and probs (P/Vg)
S_DT_NAME = "float32"
P_DT_NAME = "float32"

_cache = {}


def _build_plan(rows, cols, nb, qw):
    """Per query-window chunk lists + deduplicated key-chunk slots.

    Returns (windows, slot_blocks):
      windows: list (one per window) of chunks (slot_id, n_blocks, valid)
               where valid[kb, j] says whether key-block kb of the chunk is
               attended by query-block j of the window.
      slot_blocks: slot_id -> list of key block ids stored in that slot.
    """
    from collections import Counter

    row_cnt = [Counter() for _ in range(nb)]
    for r, c in zip(rows.tolist(), cols.tolist()):
        row_cnt[int(r)][int(c)] += 1

    slots = {}
    slot_blocks = []
    windows = []
    for w0 in range(0, nb, qw):
        cnts = [row_cnt[w0 + j] for j in range(qw)]
        cols_set = sorted(set().union(*[set(c.keys()) for c in cnts]))
        entries = []  # (block, occurrence)
        for c in cols_set:
            m = max(cnt[c] for cnt in cnts)
            entries.extend((c, k) for k in range(m))
        # maximal equal-stride runs -> chunk boundaries shared across windows
        runs = []
        i, n = 0, len(entries)
        while i < n:
            if i + 1 < n:
                stride = entries[i + 1][0] - entries[i][0]
                j = i + 1
                while j + 1 < n and entries[j + 1][0] - entries[j][0] == stride:
                    j += 1
            else:
                stride, j = 1, i
            runs.append((i, j + 1, stride))
            i = j + 1
        wchunks = []
        for a, b, stride in runs:
            for t in range(a, b, qw):
                grp = entries[t : min(t + qw, b)]
                start = grp[0][0]
                key = (start, stride if len(grp) > 1 else 1)
                sid = slots.get(key)
                if sid is None:
                    sid = len(slot_blocks)
                    slots[key] = sid
                    slot_blocks.append([])
                if len(slot_blocks[sid]) < len(grp):
                    slot_blocks[sid] = [start + key[1] * u for u in range(len(grp))]
                valid = np.ones((len(grp), qw), dtype=bool)
                for kb, (c, k) in enumerate(grp):
                    for j in range(qw):
                        valid[kb, j] = k < cnts[j][c]
                wchunks.append((sid, len(grp), valid))
        windows.append(wchunks)
    return windows, slot_blocks


def _zero_regions(valid, bs):
    """Invalid (key-block, query-block) cells as memset rectangles."""
    regs = []
    nkb, qw = valid.shape
    for kb in range(nkb):
        j = 0
        while j < qw:
            if not valid[kb, j]:
                j0 = j
                while j < qw and not valid[kb, j]:
                    j += 1
                regs.append((kb * bs, (kb + 1) * bs, j0 * bs, j * bs))
            else:
                j += 1
    return regs


def _build_nc(windows, slot_blocks, dims, s_dt_name, p_dt_name, repeat):
    import concourse.mybir as mybir
    import concourse.tile as tile
    from concourse import bacc

    S, L, HD, bs, qw, nch = dims
    s_dt = getattr(mybir.dt, s_dt_name)
    p_dt = getattr(mybir.dt, p_dt_name)
    f32 = mybir.dt.float32
    nq = qw * bs
    scale = float(HD) ** -0.5

    nc = bacc.Bacc("TRN2", debug=False)
    qt_d = nc.dram_tensor("qt", [S, HD, L], s_dt, kind="ExternalInput")
    ktg_d = nc.dram_tensor("ktg", [S, HD, nch * 128], s_dt, kind="ExternalInput")
    vg_d = nc.dram_tensor("vg", [S, 128, nch, HD + 1], p_dt, kind="ExternalInput")
    out_d = nc.dram_tensor("out", [S, L, HD], f32, kind="ExternalOutput")

    with tile.TileContext(nc) as tc:
        with (
            tc.tile_pool(name="big", bufs=2) as big,
            tc.tile_pool(name="ptp", bufs=6) as ptp,
            tc.tile_pool(name="onp", bufs=4) as onp,
            tc.tile_pool(name="stp", bufs=4, space="PSUM") as stp,
            tc.tile_pool(name="ovp", bufs=2, space="PSUM") as ovp,
        ):
            for _rep in range(repeat):
                for s in range(S):
                    qt_t = big.tile([HD, L], s_dt, tag="qt")
                    nc.sync.dma_start(out=qt_t, in_=qt_d.ap()[s])
                    ktg_t = big.tile([HD, nch * 128], s_dt, tag="ktg")
                    nc.sync.dma_start(out=ktg_t, in_=ktg_d.ap()[s])
                    vg_t = big.tile([128, nch, HD + 1], p_dt, tag="vg")
                    nc.sync.dma_start(out=vg_t, in_=vg_d.ap()[s])
                    for wi, wchunks in enumerate(windows):
                        ov_t = ovp.tile([128, HD + 1], f32, tag="ov")
                        nchunks = len(wchunks)
                        for ci, (sid, nblk, valid) in enumerate(wchunks):
                            nk = nblk * bs
                            st_t = stp.tile([128, nq], f32, tag="st")
                            nc.tensor.matmul(
                                st_t[:nk, :],
                                lhsT=ktg_t[:, sid * 128 : sid * 128 + nk],
                                rhs=qt_t[:, wi * nq : (wi + 1) * nq],
                                start=True,
                                stop=True,
                            )
                            pt_t = ptp.tile([128, nq], p_dt, tag="pt")
                            nc.scalar.activation(
                                pt_t[:nk, :],
                                st_t[:nk, :],
                                mybir.ActivationFunctionType.Exp,
                                scale=scale,
                            )
                            for p0, p1, q0, q1 in _zero_regions(valid, bs):
                                nc.vector.memset(pt_t[p0:p1, q0:q1], 0.0)
                            nc.tensor.matmul(
                                ov_t,
                                lhsT=pt_t[:nk, :],
                                rhs=vg_t[:nk, sid, :],
                                start=(ci == 0),
                                stop=(ci == nchunks - 1),
                            )
                        den_t = onp.tile([128, 1], f32, tag="den")
                        nc.vector.tensor_scalar_max(den_t, ov_t[:, HD : HD + 1], 1e-37)
                        rec_t = onp.tile([128, 1], f32, tag="rec")
                        nc.vector.reciprocal(rec_t, den_t)
                        o_t = onp.tile([128, HD], f32, tag="o")
                        nc.vector.tensor_scalar_mul(o_t, ov_t[:, 0:HD], rec_t)
                        nc.sync.dma_start(
                            out=out_d.ap()[s, wi * nq : (wi + 1) * nq, :], in_=o_t
                        )
    nc.compile()
    return nc


def _np_dt(name):
    if name == "float32":
        return np.float32
    import ml_dtypes

    return np.dtype(getattr(ml_dtypes, name))


def _prepare(query, key, value, rows, cols, block, repeat):
    B, H, L, HD = query.shape
    bs = int(block)
    nb = L // bs
    qw = max(1, 128 // bs)
    cache_key = (
        query.shape,
        bs,
        rows.tobytes(),
        cols.tobytes(),
        S_DT_NAME,
        P_DT_NAME,
        repeat,
    )
    if cache_key in _cache:
        return _cache[cache_key]

    windows, slot_blocks = _build_plan(np.asarray(rows), np.asarray(cols), nb, qw)
    nch = len(slot_blocks)
    dims = (B * H // N_CORES, L, HD, bs, qw, nch)
    nc = _build_nc(windows, slot_blocks, dims, S_DT_NAME, P_DT_NAME, repeat)
    _cache[cache_key] = (nc, windows, slot_blocks, dims)
    return _cache[cache_key]


def kernel(query, key, value, rows, cols, block):
    from concourse import bass_utils

    query = np.asarray(query)
    key = np.asarray(key)
    value = np.asarray(value)
    rows = np.asarray(rows)
    cols = np.asarray(cols)

    nc, windows, slot_blocks, dims = _prepare(
        query, key, value, rows, cols, block, repeat=1
    )
    S, L, HD, bs, qw, nch = dims
    B, H = query.shape[0], query.shape[1]
    BH = B * H
    s_np = _np_dt(S_DT_NAME)
    p_np = _np_dt(P_DT_NAME)

    q2 = query.reshape(BH, L, HD)
    k2 = key.reshape(BH, L, HD)
    v2 = value.reshape(BH, L, HD)
    qt = np.ascontiguousarray(q2.transpose(0, 2, 1)).astype(s_np)
    ktg = np.zeros((BH, HD, nch, 128), s_np)
    vg = np.zeros((BH, 128, nch, HD + 1), p_np)
    for sid, blocks in enumerate(slot_blocks):
        for kb, c in enumerate(blocks):
            kblk = k2[:, c * bs : (c + 1) * bs, :]
            ktg[:, :, sid, kb * bs : (kb + 1) * bs] = kblk.transpose(0, 2, 1)
            vg[:, kb * bs : (kb + 1) * bs, sid, :HD] = v2[:, c * bs : (c + 1) * bs, :]
            vg[:, kb * bs : (kb + 1) * bs, sid, HD] = 1.0
    ktg = ktg.reshape(BH, HD, nch * 128)

    in_maps = []
    for c in range(N_CORES):
        sl = slice(c * S, (c + 1) * S)
        in_maps.append({"qt": qt[sl], "ktg": ktg[sl], "vg": vg[sl]})

    res = bass_utils.run_bass_kernel_spmd(nc, in_maps, core_ids=list(range(N_CORES)))
    out = np.stack([res.results[c]["out"] for c in range(N_CORES)])  # [8, S, L, HD]
    return out.reshape(B, H, L, HD).astype(np.float32)


# revision 5
# speedup vs baseline: 3.8236x; 3.8236x over previous
"""Block-sparse (DeepSpeed fixed-layout) self-attention on 8 Trainium2 cores.

Strategy
--------
Shard the 32 (batch, head) slices across 8 cores (4 each, pure data parallel).
For each slice, queries are processed in windows of 128 rows (4 key-blocks of
32). The union of active key blocks for a window is split into "chunk slots"
of up to 128 keys; slots are deduplicated across windows (the fixed layout's
global stripe makes most slots shared). Host-side numpy pre-gathers:
  QT  [S, 64, L]            query transposed (hd on partitions)
  KTg [S, 64, nch*128]      gathered+transposed key chunks
  Vg  [S, 128, nch, 65]     gathered value chunks with a ones column
On device, per window and chunk:
  S_T  = KT_chunk.T-matmul (scores arrive keys-on-partitions: no P transpose)
  P    = exp(scale * S_T)   on ACT, straight from PSUM
  mask: memset invalid (key-block, query-block) cells to zero
  O~  += P.T @ [V | 1]      accumulated in PSUM; col 64 = softmax denominator
then O = O~[:, :64] * (1 / O~[:, 64]) and DMA out. exp() needs no max
subtraction: scores are ~N(0,1) after scaling, far from fp32 overflow.
"""

import sys

sys.path.insert(0, "/opt/trn_rl_repo")

import numpy as np

N_CORES = 8

# dtype knobs: storage+matmul dtype for scores (QT/KTg) and probs (P/Vg)
S_DT_NAME = "bfloat16"
P_DT_NAME = "bfloat16"

_cache = {}


def _build_plan(rows, cols, nb, qw):
    """Per query-window chunk lists + deduplicated key-chunk slots.

    Returns (windows, slot_blocks):
      windows: list (one per window) of chunks (slot_id, n_blocks, valid)
               where valid[kb, j] says whether key-block kb of the chunk is
               attended by query-block j of the window.
      slot_blocks: slot_id -> list of key block ids stored in that slot.
    """
    from collections import Counter

    row_cnt = [Counter() for _ in range(nb)]
    for r, c in zip(rows.tolist(), cols.tolist()):
        row_cnt[int(r)][int(c)] += 1

    slots = {}
    slot_blocks = []
    windows = []
    for w0 in range(0, nb, qw):
        cnts = [row_cnt[w0 + j] for j in range(qw)]
        cols_set = sorted(set().union(*[set(c.keys()) for c in cnts]))
        entries = []  # (block, occurrence)
        for c in cols_set:
            m = max(cnt[c] for cnt in cnts)
            entries.extend((c, k) for k in range(m))
        # maximal equal-stride runs -> chunk boundaries shared across windows
        runs = []
        i, n = 0, len(entries)
        while i < n:
            if i + 1 < n:
                stride = entries[i + 1][0] - entries[i][0]
                j = i + 1
                while j + 1 < n and entries[j + 1][0] - entries[j][0] == stride:
                    j += 1
            else:
                stride, j = 1, i
            runs.append((i, j + 1, stride))
            i = j + 1
        wchunks = []
        for a, b, stride in runs:
            for t in range(a, b, qw):
                grp = entries[t : min(t + qw, b)]
                start = grp[0][0]
                key = (start, stride if len(grp) > 1 else 1)
                sid = slots.get(key)
                if sid is None:
                    sid = len(slot_blocks)
                    slots[key] = sid
                    slot_blocks.append([])
                if len(slot_blocks[sid]) < len(grp):
                    slot_blocks[sid] = [start + key[1] * u for u in range(len(grp))]
                valid = np.ones((len(grp), qw), dtype=bool)
                for kb, (c, k) in enumerate(grp):
                    for j in range(qw):
                        valid[kb, j] = k < cnts[j][c]
                wchunks.append((sid, len(grp), valid))
        windows.append(wchunks)
    return windows, slot_blocks


def _zero_regions(valid, bs):
    """Invalid (key-block, query-block) cells as memset rectangles."""
    regs = []
    nkb, qw = valid.shape
    for kb in range(nkb):
        j = 0
        while j < qw:
            if not valid[kb, j]:
                j0 = j
                while j < qw and not valid[kb, j]:
                    j += 1
                regs.append((kb * bs, (kb + 1) * bs, j0 * bs, j * bs))
            else:
                j += 1
    return regs


def _build_nc(windows, slot_blocks, dims, s_dt_name, p_dt_name, repeat):
    import concourse.mybir as mybir
    import concourse.tile as tile
    from concourse import bacc

    S, L, HD, bs, qw, nch = dims
    s_dt = getattr(mybir.dt, s_dt_name)
    p_dt = getattr(mybir.dt, p_dt_name)
    f32 = mybir.dt.float32
    nq = qw * bs
    scale = float(HD) ** -0.5

    nc = bacc.Bacc("TRN2", debug=False)
    # dummy repeat-sized input: makes each repeat-variant's HLO structurally
    # unique so the neuron compile cache cannot alias them
    rtag_d = nc.dram_tensor("rtag", [1, 16 * repeat], mybir.dt.float32,
                            kind="ExternalInput")
    qt_d = nc.dram_tensor("qt", [S, HD, L], s_dt, kind="ExternalInput")
    ktg_d = nc.dram_tensor("ktg", [S, HD, nch * 128], s_dt, kind="ExternalInput")
    vg_d = nc.dram_tensor("vg", [S, 128, nch, HD + 1], p_dt, kind="ExternalInput")
    out_d = nc.dram_tensor("out", [S, L, HD], f32, kind="ExternalOutput")

    with tile.TileContext(nc) as tc:
        with (
            tc.tile_pool(name="big", bufs=2) as big,
            tc.tile_pool(name="ptp", bufs=6) as ptp,
            tc.tile_pool(name="onp", bufs=4) as onp,
            tc.tile_pool(name="stp", bufs=4, space="PSUM") as stp,
            tc.tile_pool(name="ovp", bufs=2, space="PSUM") as ovp,
        ):
            rtag_t = big.tile([1, 16 * repeat], mybir.dt.float32, tag="rtag")
            nc.sync.dma_start(out=rtag_t, in_=rtag_d.ap())
            for _rep in range(repeat):
                for s in range(S):
                    qt_t = big.tile([HD, L], s_dt, tag="qt")
                    nc.sync.dma_start(out=qt_t, in_=qt_d.ap()[s])
                    ktg_t = big.tile([HD, nch * 128], s_dt, tag="ktg")
                    nc.sync.dma_start(out=ktg_t, in_=ktg_d.ap()[s])
                    vg_t = big.tile([128, nch, HD + 1], p_dt, tag="vg")
                    nc.sync.dma_start(out=vg_t, in_=vg_d.ap()[s])
                    for wi, wchunks in enumerate(windows):
                        ov_t = ovp.tile([128, HD + 1], f32, tag="ov")
                        nchunks = len(wchunks)
                        for ci, (sid, nblk, valid) in enumerate(wchunks):
                            nk = nblk * bs
                            st_t = stp.tile([128, nq], f32, tag="st")
                            nc.tensor.matmul(
                                st_t[:nk, :],
                                lhsT=ktg_t[:, sid * 128 : sid * 128 + nk],
                                rhs=qt_t[:, wi * nq : (wi + 1) * nq],
                                start=True,
                                stop=True,
                            )
                            pt_t = ptp.tile([128, nq], p_dt, tag="pt")
                            nc.scalar.activation(
                                pt_t[:nk, :],
                                st_t[:nk, :],
                                mybir.ActivationFunctionType.Exp,
                                scale=scale,
                            )
                            for p0, p1, q0, q1 in _zero_regions(valid, bs):
                                nc.vector.memset(pt_t[p0:p1, q0:q1], 0.0)
                            nc.tensor.matmul(
                                ov_t,
                                lhsT=pt_t[:nk, :],
                                rhs=vg_t[:nk, sid, :],
                                start=(ci == 0),
                                stop=(ci == nchunks - 1),
                            )
                        den_t = onp.tile([128, 1], f32, tag="den")
                        nc.vector.tensor_scalar_max(den_t, ov_t[:, HD : HD + 1], 1e-37)
                        rec_t = onp.tile([128, 1], f32, tag="rec")
                        nc.vector.reciprocal(rec_t, den_t)
                        o_t = onp.tile([128, HD], f32, tag="o")
                        nc.vector.tensor_scalar_mul(o_t, ov_t[:, 0:HD], rec_t)
                        nc.sync.dma_start(
                            out=out_d.ap()[s, wi * nq : (wi + 1) * nq, :], in_=o_t
                        )
    nc.compile()
    return nc


def _np_dt(name):
    if name == "float32":
        return np.float32
    import ml_dtypes

    return np.dtype(getattr(ml_dtypes, name))


def _prepare(query, key, value, rows, cols, block, repeat):
    B, H, L, HD = query.shape
    bs = int(block)
    nb = L // bs
    qw = max(1, 128 // bs)
    cache_key = (
        query.shape,
        bs,
        rows.tobytes(),
        cols.tobytes(),
        S_DT_NAME,
        P_DT_NAME,
        repeat,
    )
    if cache_key in _cache:
        return _cache[cache_key]

    windows, slot_blocks = _build_plan(np.asarray(rows), np.asarray(cols), nb, qw)
    nch = len(slot_blocks)
    dims = (B * H // N_CORES, L, HD, bs, qw, nch)
    nc = _build_nc(windows, slot_blocks, dims, S_DT_NAME, P_DT_NAME, repeat)
    _cache[cache_key] = (nc, windows, slot_blocks, dims)
    return _cache[cache_key]


def kernel(query, key, value, rows, cols, block):
    from concourse import bass_utils

    query = np.asarray(query)
    key = np.asarray(key)
    value = np.asarray(value)
    rows = np.asarray(rows)
    cols = np.asarray(cols)

    nc, windows, slot_blocks, dims = _prepare(
        query, key, value, rows, cols, block, repeat=1
    )
    S, L, HD, bs, qw, nch = dims
    B, H = query.shape[0], query.shape[1]
    BH = B * H
    s_np = _np_dt(S_DT_NAME)
    p_np = _np_dt(P_DT_NAME)

    q2 = query.reshape(BH, L, HD)
    k2 = key.reshape(BH, L, HD)
    v2 = value.reshape(BH, L, HD)
    qt = np.ascontiguousarray(q2.transpose(0, 2, 1)).astype(s_np)
    ktg = np.zeros((BH, HD, nch, 128), s_np)
    vg = np.zeros((BH, 128, nch, HD + 1), p_np)
    for sid, blocks in enumerate(slot_blocks):
        for kb, c in enumerate(blocks):
            kblk = k2[:, c * bs : (c + 1) * bs, :]
            ktg[:, :, sid, kb * bs : (kb + 1) * bs] = kblk.transpose(0, 2, 1)
            vg[:, kb * bs : (kb + 1) * bs, sid, :HD] = v2[:, c * bs : (c + 1) * bs, :]
            vg[:, kb * bs : (kb + 1) * bs, sid, HD] = 1.0
    ktg = ktg.reshape(BH, HD, nch * 128)

    rtag = np.zeros((1, 16), np.float32)
    in_maps = []
    for c in range(N_CORES):
        sl = slice(c * S, (c + 1) * S)
        in_maps.append({"qt": qt[sl], "ktg": ktg[sl], "vg": vg[sl], "rtag": rtag})

    res = bass_utils.run_bass_kernel_spmd(nc, in_maps, core_ids=list(range(N_CORES)))
    out = np.stack([res.results[c]["out"] for c in range(N_CORES)])  # [8, S, L, HD]
    return out.reshape(B, H, L, HD).astype(np.float32)
